# revision 5
# baseline (speedup 1.0000x reference)
"""Multi-head attention (QKV proj + RoPE + softmax attention + output proj)
for Trainium2, tensor-parallel over heads across 8 NeuronCores.

Shapes (hardcoded): hidden_states [2, 2048, 2048], 16 heads x 128 head_dim.
Each core computes 2 heads end-to-end:
  q/k/v column-sharded projections -> RoPE -> scores^T -> exp (no max-sub,
  scores are ~+-7) -> column-sum via ones-matmul -> out^T = v^T @ exp^T ->
  normalize -> row-sharded O-projection partial.
Host sums the 8 partial outputs.

Device layouts:
  - X^T [2048 hidden, 4096 tokens] streamed as fp32r (full-rate PE).
  - q^T/k^T kept [128 d, tokens] per head (contraction on partitions).
  - v kept token-major [tokens, 256] (keys on partitions for out^T matmul).
  - RoPE via sign-folded permutation matmul: tmp = S @ q, then
    q_rot = q*cos + tmp*sin elementwise on DVE.
"""

import math

import numpy as np

HIDDEN = 2048
NH = 16
HD = 128
B = 2
S = 2048
T = B * S
NCORES = 8
HPC = NH // NCORES  # heads per core
CW = HPC * HD  # per-core projection width (256)
BASE = 10000.0
TB = 256  # phase-A token block
QB = 512  # phase-B query block
NKT = S // 128  # key tiles per batch (16)
NCH = HIDDEN // 128  # contraction chunks (16)

_CACHE = {}


def _kernel_body(tc, aps, repeat=1):
    import concourse.bass as bass  # noqa: F401
    from concourse import mybir

    nc = tc.nc
    f32 = mybir.dt.float32
    f32r = mybir.dt.float32r
    bf16 = mybir.dt.bfloat16
    Act = mybir.ActivationFunctionType

    xt_r = aps["xt"].rearrange("(c p) t -> p c t", p=128)
    wq_r = aps["wq"].rearrange("(c p) m -> p c m", p=128)
    wk_r = aps["wk"].rearrange("(c p) m -> p c m", p=128)
    wv_r = aps["wv"].rearrange("(c p) m -> p c m", p=128)
    wo_r = aps["wo"].rearrange("(h p) n -> p h n", p=128)
    out_ap = aps["out"]

    qscale = 1.0 / math.sqrt(HD)

    with (
        tc.tile_pool(name="consts", bufs=1) as consts,
        tc.tile_pool(name="big", bufs=1) as big,
        tc.tile_pool(name="xt", bufs=2) as xtp,
        tc.tile_pool(name="rope", bufs=4) as rope,
        tc.tile_pool(name="expp", bufs=1) as expp,
        tc.tile_pool(name="small", bufs=2) as small,
        tc.tile_pool(name="stage", bufs=4) as stagep,
        tc.tile_pool(name="ps", bufs=7, space="PSUM") as psp,
    ):
        # ---- constants ----
        wq_sb = consts.tile([128, NCH, CW], f32r, tag="wq")
        wk_sb = consts.tile([128, NCH, CW], f32r, tag="wk")
        wv_sb = consts.tile([128, NCH, CW], f32r, tag="wv")
        wo_sb = consts.tile([128, HPC, HIDDEN], f32r, tag="wo")
        cos_sb = consts.tile([128, S], f32, tag="cos")
        sin_sb = consts.tile([128, S], f32, tag="sin")
        st_sb = consts.tile([128, 128], f32r, tag="st")
        ones_sb = consts.tile([128, 1], bf16, tag="ones")
        bqk_sb = consts.tile([128, 4], f32, tag="bqk")
        bvb_sb = consts.tile([128, CW], f32, tag="bvb")
        nc.sync.dma_start(out=wq_sb, in_=wq_r)
        nc.sync.dma_start(out=wk_sb, in_=wk_r)
        nc.sync.dma_start(out=wv_sb, in_=wv_r)
        nc.sync.dma_start(out=wo_sb, in_=wo_r)
        nc.sync.dma_start(out=cos_sb, in_=aps["cosT"])
        nc.sync.dma_start(out=sin_sb, in_=aps["sinT"])
        nc.sync.dma_start(out=st_sb, in_=aps["st"])
        nc.sync.dma_start(out=ones_sb, in_=aps["ones"])
        nc.sync.dma_start(out=bqk_sb, in_=aps["bqk"])
        nc.sync.dma_start(out=bvb_sb, in_=aps["bvb"])

        def body(_=None):
            for b in range(B):
                # ---------- phase A: projections + RoPE ----------
                qT = big.tile([128, HPC, S], f32r, tag="qT")
                kT = big.tile([128, HPC, S], f32r, tag="kT")
                vtok = big.tile([128, NKT, CW], bf16, tag="vtok")
                for tbl in range(S // TB):
                    g0 = b * S + tbl * TB
                    s0 = tbl * TB
                    xt_t = xtp.tile([128, NCH, TB], f32r, tag="xt")
                    nc.sync.dma_start(out=xt_t, in_=xt_r[:, :, g0 : g0 + TB])
                    for h in range(HPC):
                        for qk, w_sb, bcol, scl, dstT in (
                            (0, wq_sb, h, qscale, qT),
                            (1, wk_sb, 2 + h, 1.0, kT),
                        ):
                            ps = psp.tile([128, TB], f32, tag="ps")
                            for c in range(NCH):
                                nc.tensor.matmul(
                                    ps,
                                    lhsT=w_sb[:, c, h * HD : (h + 1) * HD],
                                    rhs=xt_t[:, c, :],
                                    start=(c == 0),
                                    stop=(c == NCH - 1),
                                )
                            strt = rope.tile([128, TB], f32r, tag="rt")
                            nc.scalar.activation(
                                strt, ps, Act.Identity,
                                bias=bqk_sb[:, bcol : bcol + 1], scale=scl,
                            )
                            tps = psp.tile([128, TB], f32, tag="ps")
                            nc.tensor.matmul(tps, lhsT=st_sb, rhs=strt,
                                             start=True, stop=True)
                            t1 = rope.tile([128, TB], f32r, tag="rt")
                            nc.vector.tensor_mul(t1, strt, cos_sb[:, s0 : s0 + TB])
                            t2 = rope.tile([128, TB], f32r, tag="rt")
                            nc.vector.tensor_mul(t2, tps, sin_sb[:, s0 : s0 + TB])
                            nc.vector.tensor_add(
                                dstT[:, h, s0 : s0 + TB], t1, t2
                            )
                    for sub in range(TB // 128):
                        psv = psp.tile([128, CW], f32, tag="ps")
                        for c in range(NCH):
                            nc.tensor.matmul(
                                psv,
                                lhsT=xt_t[:, c, sub * 128 : (sub + 1) * 128],
                                rhs=wv_sb[:, c, :],
                                start=(c == 0),
                                stop=(c == NCH - 1),
                            )
                        nc.vector.tensor_add(
                            vtok[:, tbl * (TB // 128) + sub, :], psv, bvb_sb
                        )

                # ---------- phase B: attention ----------
                outT = big.tile([128, HPC, S], f32r, tag="outT")
                for h in range(HPC):
                    for qb in range(S // QB):
                        q0 = qb * QB
                        expT = expp.tile([128, NKT, QB], bf16, tag="expT")
                        for kt in range(NKT):
                            ps = psp.tile([128, QB], f32, tag="ps")
                            nc.tensor.matmul(
                                ps,
                                lhsT=kT[:, h, kt * 128 : (kt + 1) * 128],
                                rhs=qT[:, h, q0 : q0 + QB],
                                start=True,
                                stop=True,
                            )
                            nc.scalar.activation(expT[:, kt, :], ps, Act.Exp)
                        pso = psp.tile([128, QB], f32, tag="ps")
                        for kt in range(NKT):
                            nc.tensor.matmul(
                                pso,
                                lhsT=vtok[:, kt, h * HD : (h + 1) * HD],
                                rhs=expT[:, kt, :],
                                start=(kt == 0),
                                stop=(kt == NKT - 1),
                            )
                        pss = psp.tile([1, QB], f32, tag="ps")
                        for kt in range(NKT):
                            nc.tensor.matmul(
                                pss,
                                lhsT=ones_sb,
                                rhs=expT[:, kt, :],
                                start=(kt == 0),
                                stop=(kt == NKT - 1),
                            )
                        rec = small.tile([1, QB], f32, tag="rec")
                        nc.vector.reciprocal(rec, pss)
                        rbc = small.tile([128, QB], f32, tag="rec")
                        nc.gpsimd.partition_broadcast(rbc, rec)
                        nc.vector.tensor_mul(outT[:, h, q0 : q0 + QB], pso, rbc)

                # ---------- phase C: output projection ----------
                for tt in range(S // 128):
                    r0 = b * S + tt * 128
                    for nb in range(HIDDEN // QB):
                        psn = psp.tile([128, QB], f32, tag="ps")
                        for h in range(HPC):
                            nc.tensor.matmul(
                                psn,
                                lhsT=outT[:, h, tt * 128 : (tt + 1) * 128],
                                rhs=wo_sb[:, h, nb * QB : (nb + 1) * QB],
                                start=(h == 0),
                                stop=(h == HPC - 1),
                            )
                        stage = stagep.tile([128, QB], f32, tag="stage")
                        if nb % 2 == 0:
                            nc.scalar.copy(stage, psn)
                        else:
                            nc.vector.tensor_copy(stage, psn)
                        nc.sync.dma_start(
                            out=out_ap[r0 : r0 + 128, nb * QB : (nb + 1) * QB],
                            in_=stage,
                        )

        if repeat == 1:
            body()
        else:
            tc.For_i_unrolled(0, repeat, 1, body, max_unroll=1)


def _build(repeat=1):
    key = ("nc", repeat)
    if key in _CACHE:
        return _CACHE[key]
    import concourse.bacc as bacc
    import concourse.tile as tile
    from concourse import mybir

    f32 = mybir.dt.float32
    f32r = mybir.dt.float32r
    bf16 = mybir.dt.bfloat16

    nc = bacc.Bacc("TRN2", target_bir_lowering=False, debug=False)
    specs = [
        ("xt", [HIDDEN, T], f32r, "ExternalInput"),
        ("wq", [HIDDEN, CW], f32r, "ExternalInput"),
        ("wk", [HIDDEN, CW], f32r, "ExternalInput"),
        ("wv", [HIDDEN, CW], f32r, "ExternalInput"),
        ("wo", [CW, HIDDEN], f32r, "ExternalInput"),
        ("bqk", [128, 4], f32, "ExternalInput"),
        ("bvb", [128, CW], f32, "ExternalInput"),
        ("cosT", [128, S], f32, "ExternalInput"),
        ("sinT", [128, S], f32, "ExternalInput"),
        ("st", [128, 128], f32r, "ExternalInput"),
        ("ones", [128, 1], bf16, "ExternalInput"),
        ("out", [T, HIDDEN], f32, "ExternalOutput"),
    ]
    aps = {}
    for name, shape, dt_, kind in specs:
        aps[name] = nc.dram_tensor(name, shape, dt_, kind=kind).ap()
    with tile.TileContext(nc) as tc:
        _kernel_body(tc, aps, repeat=repeat)
    nc.compile()
    _CACHE[key] = nc
    return nc


def _host_inputs(hidden_states, Wq, bq, Wk, bk, Wv, bv, Wo):
    import ml_dtypes

    X = np.ascontiguousarray(
        np.asarray(hidden_states, dtype=np.float32).reshape(T, HIDDEN)
    )
    XT = np.ascontiguousarray(X.T)

    inv = 1.0 / (BASE ** (np.arange(0, HD, 2, dtype=np.float32) / HD))
    t = np.arange(S, dtype=np.float32)
    freqs = np.outer(t, inv)  # [S, 64]
    emb = np.concatenate([freqs, freqs], axis=-1)  # [S, 128]
    cosT = np.ascontiguousarray(np.cos(emb).T.astype(np.float32))  # [128, S]
    sinT = np.ascontiguousarray(np.sin(emb).T.astype(np.float32))

    # S matrix: tmp = S_ @ q gives tmp[p] = -q[p+64] (p<64), q[p-64] (p>=64)
    # matmul computes lhsT.T @ rhs, so pass st = S_^T.
    S_ = np.zeros((128, 128), dtype=np.float32)
    for p in range(64):
        S_[p, p + 64] = -1.0
        S_[p + 64, p] = 1.0
    st = np.ascontiguousarray(S_.T)

    ones = np.ones((128, 1), dtype=ml_dtypes.bfloat16)

    in_maps = []
    for c in range(NCORES):
        j0 = c * CW
        bq_c = np.asarray(bq[j0 : j0 + CW], dtype=np.float32)
        bk_c = np.asarray(bk[j0 : j0 + CW], dtype=np.float32)
        bv_c = np.asarray(bv[j0 : j0 + CW], dtype=np.float32)
        # ACT computes in*scale + bias, so pre-scale the q bias columns
        qs = 1.0 / math.sqrt(HD)
        bqk = np.stack(
            [bq_c[:HD] * qs, bq_c[HD:] * qs, bk_c[:HD], bk_c[HD:]], axis=1
        ).astype(np.float32)  # [128, 4]
        in_maps.append(
            {
                "xt": XT,
                "wq": np.ascontiguousarray(Wq[:, j0 : j0 + CW], dtype=np.float32),
                "wk": np.ascontiguousarray(Wk[:, j0 : j0 + CW], dtype=np.float32),
                "wv": np.ascontiguousarray(Wv[:, j0 : j0 + CW], dtype=np.float32),
                "wo": np.ascontiguousarray(Wo[j0 : j0 + CW, :], dtype=np.float32),
                "bqk": np.ascontiguousarray(bqk),
                "bvb": np.ascontiguousarray(
                    np.tile(bv_c[None, :], (128, 1)).astype(np.float32)
                ),
                "cosT": cosT,
                "sinT": sinT,
                "st": st,
                "ones": ones,
            }
        )
    return in_maps


def kernel(hidden_states, Wq, bq, Wk, bk, Wv, bv, Wo):
    from concourse import bass_utils

    nc = _build(repeat=1)
    in_maps = _host_inputs(hidden_states, Wq, bq, Wk, bk, Wv, bv, Wo)
    res = bass_utils.run_bass_kernel_spmd(nc, in_maps, core_ids=list(range(NCORES)))
    acc = res.results[0]["out"].astype(np.float32)
    for c in range(1, NCORES):
        acc = acc + res.results[c]["out"]
    return acc.reshape(B, S, HIDDEN)
